# revision 38
# baseline (speedup 1.0000x reference)
"""Trainium2 Bass kernel for Mixtral-style attention (B=2, S=2048, 32 q / 8 kv heads, D=128).

Sharding: 2-way data parallel over batch x 4-way tensor parallel over heads
(8 cores). Each core computes QKV projection for its head shard, RoPE, causal
GQA attention, and a partial o_proj (row-sharded). Host sums the 4 partials
per batch element.

All heavy matmuls run in bf16 with fp32 PSUM accumulation. Attention scores
are computed directly transposed (kT_blk^T @ qT_chunk) so exp(PSUM)->SBUF
lands straight in the probsT layout the attnT matmul needs; the causal mask
is a transposed-tril multiply on the diagonal 128x128 block only.

Softmax denominators stay OFF the PE streaming path: the DVE keeps a running
fp16 column-accumulator of the exp'd slab blocks, and a single ones[128,128]
matmul per (head, chunk) both partition-reduces the accumulator and
broadcasts the denominator to all 128 partitions in one 512-col pass
(replacing the per-block ones-row matmuls + reciprocal broadcast of the
previous version, ~85us of PE time). Phase B is a 3-stage software pipeline
(scores(k) | den+attnV(k-1) | epilogue(k-2)).

Phase C rotates PSUM across 6 banks and issues output-store DMA triggers
from the idle GpSimd queue so the Sync engine's ~600ns-per-DMA issue cost
never backs up the PSUM drain chain.
"""

import os
import sys

import numpy as np

for _p in ("/opt/trn_rl_repo", "/root/.axon_site/_ro/trn_rl_repo"):
    if os.path.isdir(_p) and _p not in sys.path:
        sys.path.insert(0, _p)

import ml_dtypes  # noqa: E402

import concourse.bass as bass  # noqa: E402
import concourse.mybir as mybir  # noqa: E402
import concourse.tile as tile  # noqa: E402
from concourse import bacc, bass_utils  # noqa: E402

BF16 = ml_dtypes.bfloat16
F32 = mybir.dt.float32
BF = mybir.dt.bfloat16
FP16 = mybir.dt.float16

B, S, HIDDEN = 2, 2048, 4096
NH, NKV, D = 32, 8, 128
TP, DP = 4, 2  # head-parallel x batch-parallel = 8 cores
QH = NH // TP  # 8 q heads per core
KH = NKV // TP  # 2 kv heads per core
NC_TILES = QH + 2 * KH  # 12 c-tiles of 128 per core (q..., k..., v...)
SC = 512  # s-chunk for phase A / attnT free dim
NSC = S // SC  # 4
NBLK = S // 128  # 16
ROPE_THETA = 10000.0
SM_SCALE = float(D) ** -0.5


def _emit(nc: bass.Bass):
    hT = nc.dram_tensor("hT", [128, HIDDEN // 128, S], BF, kind="ExternalInput")
    wq = nc.dram_tensor("wq", [NC_TILES, 128, 32 * 128], BF, kind="ExternalInput")
    wo = nc.dram_tensor("wo", [8, 128, 8 * 512], BF, kind="ExternalInput")
    cosT = nc.dram_tensor("cosT", [128, S], BF, kind="ExternalInput")
    sinT = nc.dram_tensor("sinT", [128, S], BF, kind="ExternalInput")
    triuD = nc.dram_tensor("triuD", [128, 128], FP16, kind="ExternalInput")
    onesD = nc.dram_tensor("onesD", [1, 128], BF, kind="ExternalInput")
    onesMD = nc.dram_tensor("onesMD", [128, 128], FP16, kind="ExternalInput")
    identD = nc.dram_tensor("identD", [128, 128], FP16, kind="ExternalInput")
    out = nc.dram_tensor("out", [S, HIDDEN], F32, kind="ExternalOutput")

    with tile.TileContext(nc) as tc:
        with (
            tc.tile_pool(name="const", bufs=1) as constp,
            tc.tile_pool(name="big", bufs=2) as bigp,
            tc.tile_pool(name="wt", bufs=3) as wtp,
            tc.tile_pool(name="pers", bufs=1) as pers,
            tc.tile_pool(name="rope", bufs=1) as ropep,
            tc.tile_pool(name="small", bufs=2) as smallp,
            tc.tile_pool(name="acc", bufs=2) as accp,
            tc.tile_pool(name="outp", bufs=4) as outp,
            tc.tile_pool(name="psum", bufs=2, space="PSUM") as psum,
            tc.tile_pool(name="psum_s", bufs=2, space="PSUM") as psum_s,
        ):
            # ---- startup: critical-path DMAs first, then constants ----
            ones1 = constp.tile([1, 128], BF, tag="ones1")
            nc.sync.dma_start(ones1, onesD[:])

            def load_hTc(sc):
                # issued from the Activation hardware-DGE queue so these
                # triggers are never head-of-line blocked behind wct triggers
                # on the Sync queue; 4-way split to spread across DMA queues.
                t = bigp.tile([128, 32, SC], BF, tag="bigslot")
                for q in range(4):
                    nc.scalar.dma_start(
                        t[:, q * 8 : (q + 1) * 8, :],
                        hT[:, q * 8 : (q + 1) * 8, sc * SC : (sc + 1) * SC],
                    )
                return t

            def load_wct(c):
                t = wtp.tile([128, 32 * 128], BF, tag="wt")
                for q in range(4):
                    nc.sync.dma_start(
                        t[:, q * 1024 : (q + 1) * 1024],
                        wq[c, :, q * 1024 : (q + 1) * 1024],
                    )
                return t

            hTc0 = load_hTc(0)
            # cos/sin ride the ACT queue: on Sync they'd sit behind the wct
            # quarters and arrive after rope(c=0) needs them.
            cos_sb = constp.tile([128, S], BF, tag="cos")
            sin_sb = constp.tile([128, S], BF, tag="sin")
            nc.scalar.dma_start(cos_sb, cosT[:])
            nc.scalar.dma_start(sin_sb, sinT[:])
            wct_pre = [load_wct(0), load_wct(1), load_wct(2)]

            triu = constp.tile([128, 128], FP16, tag="triu")
            onesM = constp.tile([128, 128], FP16, tag="onesM")
            ident = constp.tile([128, 128], FP16, tag="ident")
            nc.sync.dma_start(triu, triuD[:])
            nc.sync.dma_start(onesM, onesMD[:])
            nc.sync.dma_start(ident, identD[:])

            # HAM warm-up: dummy matmuls on the tiny ones row while the first
            # hidden/weight DMAs are in flight, so the PE is already
            # un-throttled when real data arrives.
            wps = psum_s.tile([128, 2, 512], F32, tag="scores")
            for w in range(32):
                nc.tensor.matmul(
                    wps[:, 0, :128], ones1, ones1, start=(w == 0), stop=(w == 31),
                    skip_group_check=True,
                )
            dwarm = smallp.tile([128, 128], BF, tag="dwarm")
            nc.scalar.copy(dwarm, wps[:, 0, :128])

            # persistent activations
            qT = pers.tile([128, QH, S], BF, tag="qT")  # [d, head, s]
            kT = pers.tile([128, KH, S], BF, tag="kT")
            vN = pers.tile([128, KH * NBLK, 128], FP16, tag="vN")  # [sk, kv*blk, d]
            aT = pers.tile([128, QH, S], BF, tag="aT")  # [d, head, s]

            def rope_into(dst, ps, sc):
                # dst = ps * cos + rot(ps) * sin ; rot = [-x2, x1]
                rot = ropep.tile([128, SC], F32, tag="rot")
                nc.scalar.mul(rot[0:64, :], ps[64:128, :], -1.0)
                nc.scalar.copy(rot[64:128, :], ps[0:64, :])
                t2 = ropep.tile([128, SC], F32, tag="t2")
                cs = cos_sb[:, sc * SC : (sc + 1) * SC]
                sn = sin_sb[:, sc * SC : (sc + 1) * SC]
                nc.vector.tensor_mul(t2, ps, cs)
                nc.vector.tensor_mul(rot, rot, sn)
                nc.vector.tensor_add(dst, t2, rot)

            # ---- Phase B: causal GQA attention per head ----
            # slab[:, j, :] holds (unnormalized) probsT for sk-block j of the
            # current sq-chunk, in fp16. As each block is exp'd the DVE folds
            # it into a running fp16 accumulator `acc`; one ones[128,128]
            # matmul per (h, m) then partition-reduces acc AND broadcasts the
            # denominator to all 128 partitions; reciprocal + normalize are
            # DVE-only.
            def b_scores(h, m):
                kv = h // (QH // KH)
                if m == 0:
                    # m=0 slabs are tiny and come from their own pool so these
                    # stages can interleave into phase A's last chunk while
                    # bigp's two buffers still hold hTc tiles.
                    slab = accp.tile([128, 4, SC], FP16, tag="slab0")
                else:
                    slab = bigp.tile([128, NBLK, SC], FP16, tag="bigslot")
                acc = accp.tile([128, SC], FP16, tag="acc")
                qm = qT[:, h, m * 512 : (m + 1) * 512]
                for p in range(2 * m + 2):  # block pairs (2p, 2p+1)
                    j0 = 2 * p
                    diag = j0 >= 4 * m
                    pps = psum_s.tile([128, 2, 512], F32, tag="scores")
                    for u in range(2):
                        j = j0 + u
                        c0 = max(0, j - 4 * m) * 128
                        # diagonal blocks write at their ALIGNED offset so a
                        # single fused exp covers the pair; the dead columns
                        # [0, c0) hold garbage that no consumer ever reads.
                        nc.tensor.matmul(
                            pps[:, u, c0:],
                            kT[:, kv, j * 128 : (j + 1) * 128],
                            qm[:, c0:],
                            start=True,
                            stop=True,
                            skip_group_check=True,
                        )
                    nc.scalar.activation(
                        slab[:, j0 : j0 + 2, :],
                        pps,
                        mybir.ActivationFunctionType.Exp,
                        scale=SM_SCALE,
                    )
                    for u in range(2):
                        j = j0 + u
                        c0 = max(0, j - 4 * m) * 128
                        if diag:
                            blk = slab[:, j, c0 : c0 + 128]
                            nc.vector.tensor_mul(blk, blk, triu)
                        if j == 0:
                            nc.vector.tensor_copy(acc, slab[:, 0, :])
                        else:
                            nc.vector.tensor_add(
                                acc[:, c0:], acc[:, c0:], slab[:, j, c0:]
                            )
                return slab, acc

            def b_den(h, m, acc):
                # den matmul + reciprocal, emitted BEFORE the next stage's
                # DVE add-chain so the reciprocal (and the trailing epilogue
                # mul) are not queued behind ~5us of adds on the in-order DVE.
                dps = psum.tile([128, 512], F32, tag="mm512")
                nc.tensor.matmul(dps, onesM, acc, start=True, stop=True)
                rcpb = smallp.tile([128, 512], F32, tag="rcpb")
                nc.vector.reciprocal_approx_fast(rcpb, dps)
                return rcpb

            def b_attnv(h, m, slab):
                kv = h // (QH // KH)
                aps = psum.tile([128, 512], F32, tag="attn")
                for j in range(4 * m):
                    nc.tensor.matmul(
                        aps, vN[:, kv * NBLK + j, :], slab[:, j, :],
                        start=(j == 0), stop=False, skip_group_check=True,
                    )
                for jj in range(4):
                    j = 4 * m + jj
                    cs = slice(jj * 128, 512)
                    first = m == 0 and jj == 0
                    nc.tensor.matmul(
                        aps[:, cs], vN[:, kv * NBLK + j, :], slab[:, j, cs],
                        start=first, stop=(jj == 3), skip_group_check=True,
                    )
                return aps

            def b_epilogue(h, m, aps, rcpb):
                nc.vector.tensor_mul(aT[:, h, m * 512 : (m + 1) * 512], aps, rcpb)

            # ---- Phase C tiles (partial o_proj = attnT^T @ w_o_shard), ----
            # interleaved into phase B as PE filler. Once all 8 heads of
            # sequence chunk m have been normalized into aT, the 32 o_proj
            # tiles for st in [4m, 4m+4) are pushed onto a queue and drained
            # a few per pipeline stage while the next chunk's attention is
            # ACT/DVE-bound.
            from collections import deque

            c_pending = deque()
            wot_cur = {}

            def load_wot(hc):
                t = wtp.tile([128, 8 * 512], BF, tag="wt")
                for q in range(4):
                    nc.sync.dma_start(
                        t[:, q * 1024 : (q + 1) * 1024],
                        wo[hc, :, q * 1024 : (q + 1) * 1024],
                    )
                return t

            def push_c_group(m):
                # wot markers one quad early so weights prefetch ~7us ahead
                c_pending.append(("wot", 0))
                for hc in range(8):
                    if hc + 1 < 8:
                        c_pending.append(("wot", hc + 1))
                    for st in range(4 * m, 4 * m + 4):
                        c_pending.append(("tile", hc, st))

            c_rot = [0, None]  # rotation counter / current scores pair tile

            def emit_c_tile(hc, st, tail=False):
                wot = wot_cur[hc]
                if not tail:
                    ops = psum.tile([128, 512], F32, tag="mm512")
                else:
                    # after phase B drains, rotate over all free PSUM banks
                    r = c_rot[0] % 4
                    c_rot[0] += 1
                    if r == 0:
                        ops = psum.tile([128, 512], F32, tag="mm512")
                    elif r == 1:
                        ops = psum.tile([128, 512], F32, tag="attn")
                    elif r == 2:
                        cpair = psum_s.tile([128, 2, 512], F32, tag="scores")
                        c_rot[1] = cpair
                        ops = cpair[:, 0, :]
                    else:
                        ops = c_rot[1][:, 1, :]
                for cb in range(QH):
                    nc.tensor.matmul(
                        ops,
                        aT[:, cb, st * 128 : (st + 1) * 128],
                        wot[:, cb * 512 : (cb + 1) * 512],
                        start=(cb == 0),
                        stop=(cb == QH - 1),
                    )
                ot = outp.tile([128, 512], F32, tag="ot")
                nc.scalar.copy(ot, ops)
                nc.gpsimd.dma_start(
                    out[st * 128 : (st + 1) * 128, hc * 512 : (hc + 1) * 512], ot
                )

            def pop_c(n, tail=False):
                emitted = 0
                while c_pending:
                    item = c_pending[0]
                    if item[0] == "wot":
                        # process weight-load markers eagerly (prefetch)
                        c_pending.popleft()
                        wot_cur[item[1]] = load_wot(item[1])
                        continue
                    if emitted >= n:
                        break
                    _, hc, st = c_pending.popleft()
                    emit_c_tile(hc, st, tail=tail)
                    emitted += 1

            # 3-stage software pipeline over (chunk, head). Per stage k the
            # emission order is: den(k-1)+recip(k-1), epilogue-mul(k-2) (both
            # ahead of the new DVE add-chain), scores(k), filler, attnV(k-1),
            # filler. o_proj filler tiles are popped between stages.
            # The m=0 stages are emitted by the phase A loop (interleaved into
            # sc=3, where the PE stream hides their ACT-bound exp latency).
            seq = [(h, m) for m in range(NSC) for h in range(QH)]
            bst = {"k": 0, "st1": None, "st2": None}

            def b_stage():
                h, m = seq[bst["k"]]
                bst["k"] += 1
                st1, st2 = bst["st1"], bst["st2"]
                prcpb = None
                if st1 is not None:
                    ph, pm, pslab, pacc = st1
                    prcpb = b_den(ph, pm, pacc)
                    if st2 is not None:
                        b_epilogue(*st2)
                        if st2[0] == QH - 1:  # chunk st2[1] fully in aT
                            push_c_group(st2[1])
                            pop_c(0)  # eager wot prefetch
                slab, acc = b_scores(h, m)
                pop_c(2)
                if st1 is not None:
                    ph, pm, pslab, pacc = st1
                    aps = b_attnv(ph, pm, pslab)
                    bst["st2"] = (ph, pm, aps, prcpb)
                bst["st1"] = (h, m, slab, acc)
                pop_c(2)

            # ---- Phase A: QKV^T = w_shard^T @ hidden^T, RoPE, V transpose.
            # The 8 m=0 attention stages (which only need sc=0 outputs) are
            # interleaved into sc=3 as extra PE work to hide their ACT-bound
            # exp latency.
            hTc = hTc0
            hTc_next = None
            for sc in range(NSC):
                for c in range(NC_TILES):
                    if sc == 0 and c < 3:
                        wct = wct_pre[c]
                    else:
                        wct = load_wct(c)
                    if c == 2 and sc + 1 < NSC:
                        hTc_next = load_hTc(sc + 1)
                    ps = psum.tile([128, SC], F32, tag="mm512")
                    for ho in range(32):
                        nc.tensor.matmul(
                            ps,
                            wct[:, ho * 128 : (ho + 1) * 128],
                            hTc[:, ho, :],
                            start=(ho == 0),
                            stop=(ho == 31),
                        )
                    if c < QH:
                        rope_into(qT[:, c, sc * SC : (sc + 1) * SC], ps, sc)
                    elif c < QH + KH:
                        rope_into(kT[:, c - QH, sc * SC : (sc + 1) * SC], ps, sc)
                    else:
                        kv = c - QH - KH
                        vt = ropep.tile([128, SC], FP16, tag="vt")
                        nc.scalar.copy(vt, ps)
                        # V transpose on the PE (DMA-transpose triggers cost
                        # ~1.2us of issue time each and head-of-line block
                        # whichever queue issues them). The idle "attn" PSUM
                        # slot holds the transposed blocks.
                        tps = psum.tile([128, 4, 128], FP16, tag="attn")
                        for j in range(SC // 128):
                            nc.tensor.transpose(
                                tps[:, j, :], vt[:, j * 128 : (j + 1) * 128], ident
                            )
                        b0 = kv * NBLK + sc * 4
                        nc.scalar.copy(vN[:, b0 : b0 + 4, :], tps)
                    if sc == NSC - 1 and c >= NC_TILES - QH:
                        b_stage()  # m=0 attention stages ride along
                hTc = hTc_next

            # ---- remaining pipeline stages, drain, and o_proj tail ----
            while bst["k"] < len(seq):
                b_stage()
            ph, pm, pslab, pacc = bst["st1"]
            prcpb = b_den(ph, pm, pacc)
            if bst["st2"] is not None:
                b_epilogue(*bst["st2"])
                if bst["st2"][0] == QH - 1:
                    push_c_group(bst["st2"][1])
            aps = b_attnv(ph, pm, pslab)
            b_epilogue(ph, pm, aps, prcpb)
            push_c_group(pm)
            pop_c(10 ** 9, tail=True)

    return nc


_CACHE = {}


def build_program():
    if "nc" not in _CACHE:
        nc = bacc.Bacc()
        _emit(nc)
        nc.compile()
        _CACHE["nc"] = nc
    return _CACHE["nc"]


def host_inputs(positions, hidden_states, w_qkv, w_o):
    """Build the 8 per-core input maps (host-side shard + layout + bf16 cast)."""
    positions = np.asarray(positions)
    hidden_states = np.asarray(hidden_states, dtype=np.float32)
    w_qkv = np.asarray(w_qkv, dtype=np.float32)
    w_o = np.asarray(w_o, dtype=np.float32)

    inv_freq = 1.0 / (
        ROPE_THETA ** (np.arange(0, D, 2, dtype=np.float32) / D)
    )  # [64]
    trium = np.triu(np.ones((128, 128), dtype=np.float32)).astype(np.float16)

    # per-batch tensors
    hTs, coss, sins = [], [], []
    for b in range(B):
        hT = (
            np.ascontiguousarray(hidden_states[b].T)  # [HIDDEN, S]
            .reshape(HIDDEN // 128, 128, S)
            .transpose(1, 0, 2)  # [128, ho, S]
        )
        hTs.append(np.ascontiguousarray(hT.astype(BF16)))
        ang = positions[b].astype(np.float32)[:, None] * inv_freq[None, :]  # [S,64]
        c = np.cos(ang).T  # [64, S]
        s = np.sin(ang).T
        coss.append(np.concatenate([c, c], axis=0).astype(BF16))
        sins.append(np.concatenate([s, s], axis=0).astype(BF16))

    in_maps = []
    for core in range(8):
        b, t = divmod(core, TP)
        qcols = w_qkv[:, t * QH * D : (t + 1) * QH * D]
        kcols = w_qkv[:, NH * D + t * KH * D : NH * D + (t + 1) * KH * D]
        vcols = w_qkv[:, (NH + NKV) * D + t * KH * D : (NH + NKV) * D + (t + 1) * KH * D]
        wshard = np.concatenate([qcols, kcols, vcols], axis=1)  # [4096, 1536]
        wq_t = (
            wshard.reshape(32, 128, NC_TILES, 128)
            .transpose(2, 1, 0, 3)  # [c, p, ho, m]
            .reshape(NC_TILES, 128, 32 * 128)
            .astype(BF16)
        )
        wo_shard = w_o[t * QH * D : (t + 1) * QH * D, :]  # [1024, 4096]
        wo_t = (
            wo_shard.reshape(QH, 128, 8, 512)
            .transpose(2, 1, 0, 3)  # [hc, p, co, n]
            .reshape(8, 128, 8 * 512)
            .astype(BF16)
        )
        in_maps.append(
            {
                "hT": hTs[b],
                "wq": np.ascontiguousarray(wq_t),
                "wo": np.ascontiguousarray(wo_t),
                "cosT": coss[b],
                "sinT": sins[b],
                "triuD": trium,
                "onesD": np.ones((1, 128), dtype=BF16),
                "onesMD": np.ones((128, 128), dtype=np.float16),
                "identD": np.eye(128, dtype=np.float16),
            }
        )
    return in_maps


def gather_output(results):
    """Sum the 4 TP partials per batch -> [B, S, HIDDEN] fp32."""
    outs = []
    for b in range(B):
        acc = np.zeros((S, HIDDEN), dtype=np.float32)
        for t in range(TP):
            acc += results[b * TP + t]["out"]
        outs.append(acc)
    return np.stack(outs, axis=0)


def kernel(positions, hidden_states, w_qkv, w_o, trace=False):
    nc = build_program()
    in_maps = host_inputs(positions, hidden_states, w_qkv, w_o)
    last_err = None
    for attempt in range(3):
        try:
            res = bass_utils.run_bass_kernel_spmd(
                nc, in_maps, core_ids=list(range(8)), trace=trace
            )
            break
        except Exception as e:  # transient NRT/axon device errors
            last_err = e
            import time as _time

            _time.sleep(5 * (attempt + 1))
    else:
        raise last_err
    out = gather_output(res.results)
    if trace:
        kernel.last_exec_time_ns = res.exec_time_ns
        kernel.last_results = res
    return out


# revision 39
# speedup vs baseline: 1.0135x; 1.0135x over previous
"""Trainium2 Bass kernel for Mixtral-style attention (B=2, S=2048, 32 q / 8 kv heads, D=128).

Sharding: 2-way data parallel over batch x 4-way tensor parallel over heads
(8 cores). Each core computes QKV projection for its head shard, RoPE, causal
GQA attention, and a partial o_proj (row-sharded). Host sums the 4 partials
per batch element.

All heavy matmuls run in bf16 with fp32 PSUM accumulation. Attention scores
are computed directly transposed (kT_blk^T @ qT_chunk) so exp(PSUM)->SBUF
lands straight in the probsT layout the attnT matmul needs; the causal mask
is a transposed-tril multiply on the diagonal 128x128 block only.

Softmax denominators stay OFF the PE streaming path: the DVE keeps a running
fp16 column-accumulator of the exp'd slab blocks, and a single ones[128,128]
matmul per (head, chunk) both partition-reduces the accumulator and
broadcasts the denominator to all 128 partitions in one 512-col pass
(replacing the per-block ones-row matmuls + reciprocal broadcast of the
previous version, ~85us of PE time). Phase B is a 3-stage software pipeline
(scores(k) | den+attnV(k-1) | epilogue(k-2)).

Phase C rotates PSUM across 6 banks and issues output-store DMA triggers
from the idle GpSimd queue so the Sync engine's ~600ns-per-DMA issue cost
never backs up the PSUM drain chain.
"""

import os
import sys

import numpy as np

for _p in ("/opt/trn_rl_repo", "/root/.axon_site/_ro/trn_rl_repo"):
    if os.path.isdir(_p) and _p not in sys.path:
        sys.path.insert(0, _p)

import ml_dtypes  # noqa: E402

import concourse.bass as bass  # noqa: E402
import concourse.mybir as mybir  # noqa: E402
import concourse.tile as tile  # noqa: E402
from concourse import bacc, bass_utils  # noqa: E402

BF16 = ml_dtypes.bfloat16
F32 = mybir.dt.float32
BF = mybir.dt.bfloat16
FP16 = mybir.dt.float16

B, S, HIDDEN = 2, 2048, 4096
NH, NKV, D = 32, 8, 128
TP, DP = 4, 2  # head-parallel x batch-parallel = 8 cores
QH = NH // TP  # 8 q heads per core
KH = NKV // TP  # 2 kv heads per core
NC_TILES = QH + 2 * KH  # 12 c-tiles of 128 per core (q..., k..., v...)
SC = 512  # s-chunk for phase A / attnT free dim
NSC = S // SC  # 4
NBLK = S // 128  # 16
ROPE_THETA = 10000.0
SM_SCALE = float(D) ** -0.5


def _emit(nc: bass.Bass):
    hT = nc.dram_tensor("hT", [128, HIDDEN // 128, S], BF, kind="ExternalInput")
    wq = nc.dram_tensor("wq", [NC_TILES, 128, 32 * 128], BF, kind="ExternalInput")
    wo = nc.dram_tensor("wo", [8, 128, 8 * 512], BF, kind="ExternalInput")
    cosT = nc.dram_tensor("cosT", [128, S], BF, kind="ExternalInput")
    sinT = nc.dram_tensor("sinT", [128, S], BF, kind="ExternalInput")
    triuD = nc.dram_tensor("triuD", [128, 128], FP16, kind="ExternalInput")
    onesD = nc.dram_tensor("onesD", [1, 128], BF, kind="ExternalInput")
    onesMD = nc.dram_tensor("onesMD", [128, 128], FP16, kind="ExternalInput")
    identD = nc.dram_tensor("identD", [128, 128], FP16, kind="ExternalInput")
    out = nc.dram_tensor("out", [S, HIDDEN], F32, kind="ExternalOutput")

    with tile.TileContext(nc) as tc:
        with (
            tc.tile_pool(name="const", bufs=1) as constp,
            tc.tile_pool(name="big", bufs=2) as bigp,
            tc.tile_pool(name="wt", bufs=3) as wtp,
            tc.tile_pool(name="pers", bufs=1) as pers,
            tc.tile_pool(name="rope", bufs=1) as ropep,
            tc.tile_pool(name="small", bufs=2) as smallp,
            tc.tile_pool(name="acc", bufs=2) as accp,
            tc.tile_pool(name="outp", bufs=4) as outp,
            tc.tile_pool(name="psum", bufs=2, space="PSUM") as psum,
            tc.tile_pool(name="psum_s", bufs=2, space="PSUM") as psum_s,
        ):
            # ---- startup: critical-path DMAs first, then constants ----
            ones1 = constp.tile([1, 128], BF, tag="ones1")
            nc.sync.dma_start(ones1, onesD[:])

            def load_hTc(sc, eng=None):
                # startup load rides the ACT queue (Sync is busy with wct);
                # in-loop loads ride Sync, where their ~4us strided-descriptor
                # DIRECT2D issue cannot block the rope PSUM drains on ACT.
                eng = eng or nc.sync
                t = bigp.tile([128, 32, SC], BF, tag="bigslot")
                for q in range(4):
                    eng.dma_start(
                        t[:, q * 8 : (q + 1) * 8, :],
                        hT[:, q * 8 : (q + 1) * 8, sc * SC : (sc + 1) * SC],
                    )
                return t

            def load_wct(c):
                t = wtp.tile([128, 32 * 128], BF, tag="wt")
                for q in range(4):
                    nc.sync.dma_start(
                        t[:, q * 1024 : (q + 1) * 1024],
                        wq[c, :, q * 1024 : (q + 1) * 1024],
                    )
                return t

            hTc0 = load_hTc(0, eng=nc.scalar)
            # cos/sin ride the ACT queue: on Sync they'd sit behind the wct
            # quarters and arrive after rope(c=0) needs them.
            cos_sb = constp.tile([128, S], BF, tag="cos")
            sin_sb = constp.tile([128, S], BF, tag="sin")
            nc.scalar.dma_start(cos_sb, cosT[:])
            nc.scalar.dma_start(sin_sb, sinT[:])
            wct_pre = [load_wct(0), load_wct(1), load_wct(2)]

            triu = constp.tile([128, 128], FP16, tag="triu")
            onesM = constp.tile([128, 128], FP16, tag="onesM")
            ident = constp.tile([128, 128], FP16, tag="ident")
            nc.sync.dma_start(triu, triuD[:])
            nc.sync.dma_start(onesM, onesMD[:])
            nc.sync.dma_start(ident, identD[:])

            # HAM warm-up: dummy matmuls on the tiny ones row while the first
            # hidden/weight DMAs are in flight, so the PE is already
            # un-throttled when real data arrives.
            wps = psum_s.tile([128, 2, 512], F32, tag="scores")
            for w in range(32):
                nc.tensor.matmul(
                    wps[:, 0, :128], ones1, ones1, start=(w == 0), stop=(w == 31),
                    skip_group_check=True,
                )
            dwarm = smallp.tile([128, 128], BF, tag="dwarm")
            nc.scalar.copy(dwarm, wps[:, 0, :128])

            # persistent activations
            qT = pers.tile([128, QH, S], BF, tag="qT")  # [d, head, s]
            kT = pers.tile([128, KH, S], BF, tag="kT")
            vN = pers.tile([128, KH * NBLK, 128], FP16, tag="vN")  # [sk, kv*blk, d]
            aT = pers.tile([128, QH, S], BF, tag="aT")  # [d, head, s]

            def rope_into(dst, ps, sc):
                # dst = ps * cos + rot(ps) * sin ; rot = [-x2, x1]
                rot = ropep.tile([128, SC], F32, tag="rot")
                nc.scalar.mul(rot[0:64, :], ps[64:128, :], -1.0)
                nc.scalar.copy(rot[64:128, :], ps[0:64, :])
                t2 = ropep.tile([128, SC], F32, tag="t2")
                cs = cos_sb[:, sc * SC : (sc + 1) * SC]
                sn = sin_sb[:, sc * SC : (sc + 1) * SC]
                nc.vector.tensor_mul(t2, ps, cs)
                nc.vector.tensor_mul(rot, rot, sn)
                nc.vector.tensor_add(dst, t2, rot)

            # ---- Phase B: causal GQA attention per head ----
            # slab[:, j, :] holds (unnormalized) probsT for sk-block j of the
            # current sq-chunk, in fp16. As each block is exp'd the DVE folds
            # it into a running fp16 accumulator `acc`; one ones[128,128]
            # matmul per (h, m) then partition-reduces acc AND broadcasts the
            # denominator to all 128 partitions; reciprocal + normalize are
            # DVE-only.
            def b_scores(h, m):
                kv = h // (QH // KH)
                if m == 0:
                    # m=0 slabs are tiny and come from their own pool so these
                    # stages can interleave into phase A's last chunk while
                    # bigp's two buffers still hold hTc tiles.
                    slab = accp.tile([128, 4, SC], FP16, tag="slab0")
                else:
                    slab = bigp.tile([128, NBLK, SC], FP16, tag="bigslot")
                acc = accp.tile([128, SC], FP16, tag="acc")
                qm = qT[:, h, m * 512 : (m + 1) * 512]
                for p in range(2 * m + 2):  # block pairs (2p, 2p+1)
                    j0 = 2 * p
                    diag = j0 >= 4 * m
                    pps = psum_s.tile([128, 2, 512], F32, tag="scores")
                    for u in range(2):
                        j = j0 + u
                        c0 = max(0, j - 4 * m) * 128
                        # diagonal blocks write at their ALIGNED offset so a
                        # single fused exp covers the pair; the dead columns
                        # [0, c0) hold garbage that no consumer ever reads.
                        nc.tensor.matmul(
                            pps[:, u, c0:],
                            kT[:, kv, j * 128 : (j + 1) * 128],
                            qm[:, c0:],
                            start=True,
                            stop=True,
                            skip_group_check=True,
                        )
                    nc.scalar.activation(
                        slab[:, j0 : j0 + 2, :],
                        pps,
                        mybir.ActivationFunctionType.Exp,
                        scale=SM_SCALE,
                    )
                    for u in range(2):
                        j = j0 + u
                        c0 = max(0, j - 4 * m) * 128
                        if diag:
                            blk = slab[:, j, c0 : c0 + 128]
                            nc.vector.tensor_mul(blk, blk, triu)
                        if j == 0:
                            nc.vector.tensor_copy(acc, slab[:, 0, :])
                        else:
                            nc.vector.tensor_add(
                                acc[:, c0:], acc[:, c0:], slab[:, j, c0:]
                            )
                return slab, acc

            def b_den(h, m, acc):
                # den matmul + reciprocal, emitted BEFORE the next stage's
                # DVE add-chain so the reciprocal (and the trailing epilogue
                # mul) are not queued behind ~5us of adds on the in-order DVE.
                dps = psum.tile([128, 512], F32, tag="mm512")
                nc.tensor.matmul(dps, onesM, acc, start=True, stop=True)
                rcpb = smallp.tile([128, 512], F32, tag="rcpb")
                nc.vector.reciprocal_approx_fast(rcpb, dps)
                return rcpb

            def b_attnv(h, m, slab):
                kv = h // (QH // KH)
                aps = psum.tile([128, 512], F32, tag="attn")
                for j in range(4 * m):
                    nc.tensor.matmul(
                        aps, vN[:, kv * NBLK + j, :], slab[:, j, :],
                        start=(j == 0), stop=False, skip_group_check=True,
                    )
                for jj in range(4):
                    j = 4 * m + jj
                    cs = slice(jj * 128, 512)
                    first = m == 0 and jj == 0
                    nc.tensor.matmul(
                        aps[:, cs], vN[:, kv * NBLK + j, :], slab[:, j, cs],
                        start=first, stop=(jj == 3), skip_group_check=True,
                    )
                return aps

            def b_epilogue(h, m, aps, rcpb):
                nc.vector.tensor_mul(aT[:, h, m * 512 : (m + 1) * 512], aps, rcpb)

            # ---- Phase C tiles (partial o_proj = attnT^T @ w_o_shard), ----
            # interleaved into phase B as PE filler. Once all 8 heads of
            # sequence chunk m have been normalized into aT, the 32 o_proj
            # tiles for st in [4m, 4m+4) are pushed onto a queue and drained
            # a few per pipeline stage while the next chunk's attention is
            # ACT/DVE-bound.
            from collections import deque

            c_pending = deque()
            wot_cur = {}

            def load_wot(hc):
                t = wtp.tile([128, 8 * 512], BF, tag="wt")
                for q in range(4):
                    nc.sync.dma_start(
                        t[:, q * 1024 : (q + 1) * 1024],
                        wo[hc, :, q * 1024 : (q + 1) * 1024],
                    )
                return t

            def push_c_group(m):
                # wot markers one quad early so weights prefetch ~7us ahead
                c_pending.append(("wot", 0))
                for hc in range(8):
                    if hc + 1 < 8:
                        c_pending.append(("wot", hc + 1))
                    for st in range(4 * m, 4 * m + 4):
                        c_pending.append(("tile", hc, st))

            c_rot = [0, None]  # rotation counter / current scores pair tile

            def emit_c_tile(hc, st, tail=False):
                wot = wot_cur[hc]
                if not tail:
                    ops = psum.tile([128, 512], F32, tag="mm512")
                else:
                    # after phase B drains, rotate over all free PSUM banks
                    r = c_rot[0] % 4
                    c_rot[0] += 1
                    if r == 0:
                        ops = psum.tile([128, 512], F32, tag="mm512")
                    elif r == 1:
                        ops = psum.tile([128, 512], F32, tag="attn")
                    elif r == 2:
                        cpair = psum_s.tile([128, 2, 512], F32, tag="scores")
                        c_rot[1] = cpair
                        ops = cpair[:, 0, :]
                    else:
                        ops = c_rot[1][:, 1, :]
                for cb in range(QH):
                    nc.tensor.matmul(
                        ops,
                        aT[:, cb, st * 128 : (st + 1) * 128],
                        wot[:, cb * 512 : (cb + 1) * 512],
                        start=(cb == 0),
                        stop=(cb == QH - 1),
                    )
                ot = outp.tile([128, 512], F32, tag="ot")
                nc.scalar.copy(ot, ops)
                # tail stores alternate queues so neither trigger queue's
                # issue latency paces the drain chain
                seng = nc.sync if (tail and st % 2) else nc.gpsimd
                seng.dma_start(
                    out[st * 128 : (st + 1) * 128, hc * 512 : (hc + 1) * 512], ot
                )

            def pop_c(n, tail=False):
                emitted = 0
                while c_pending:
                    item = c_pending[0]
                    if item[0] == "wot":
                        # process weight-load markers eagerly (prefetch)
                        c_pending.popleft()
                        wot_cur[item[1]] = load_wot(item[1])
                        continue
                    if emitted >= n:
                        break
                    _, hc, st = c_pending.popleft()
                    emit_c_tile(hc, st, tail=tail)
                    emitted += 1

            # 3-stage software pipeline over (chunk, head). Per stage k the
            # emission order is: den(k-1)+recip(k-1), epilogue-mul(k-2) (both
            # ahead of the new DVE add-chain), scores(k), filler, attnV(k-1),
            # filler. o_proj filler tiles are popped between stages.
            # The m=0 stages are emitted by the phase A loop (interleaved into
            # sc=3, where the PE stream hides their ACT-bound exp latency).
            seq = [(h, m) for m in range(NSC) for h in range(QH)]
            bst = {"k": 0, "st1": None, "st2": None}

            def b_stage():
                h, m = seq[bst["k"]]
                bst["k"] += 1
                st1, st2 = bst["st1"], bst["st2"]
                prcpb = None
                if st1 is not None:
                    ph, pm, pslab, pacc = st1
                    prcpb = b_den(ph, pm, pacc)
                    if st2 is not None:
                        b_epilogue(*st2)
                        if st2[0] == QH - 1:  # chunk st2[1] fully in aT
                            push_c_group(st2[1])
                            pop_c(0)  # eager wot prefetch
                slab, acc = b_scores(h, m)
                pop_c(2)
                if st1 is not None:
                    ph, pm, pslab, pacc = st1
                    aps = b_attnv(ph, pm, pslab)
                    bst["st2"] = (ph, pm, aps, prcpb)
                bst["st1"] = (h, m, slab, acc)
                pop_c(2)

            # ---- Phase A: QKV^T = w_shard^T @ hidden^T, RoPE, V transpose.
            # The 8 m=0 attention stages (which only need sc=0 outputs) are
            # interleaved into sc=3 as extra PE work to hide their ACT-bound
            # exp latency.
            hTc = hTc0
            hTc_next = None
            for sc in range(NSC):
                for c in range(NC_TILES):
                    if sc == 0 and c < 3:
                        wct = wct_pre[c]
                    else:
                        wct = load_wct(c)
                    if c == 2 and sc + 1 < NSC:
                        hTc_next = load_hTc(sc + 1)
                    ps = psum.tile([128, SC], F32, tag="mm512")
                    for ho in range(32):
                        nc.tensor.matmul(
                            ps,
                            wct[:, ho * 128 : (ho + 1) * 128],
                            hTc[:, ho, :],
                            start=(ho == 0),
                            stop=(ho == 31),
                        )
                    if c < QH:
                        rope_into(qT[:, c, sc * SC : (sc + 1) * SC], ps, sc)
                    elif c < QH + KH:
                        rope_into(kT[:, c - QH, sc * SC : (sc + 1) * SC], ps, sc)
                    else:
                        kv = c - QH - KH
                        vt = ropep.tile([128, SC], FP16, tag="vt")
                        nc.scalar.copy(vt, ps)
                        # V transpose on the PE (DMA-transpose triggers cost
                        # ~1.2us of issue time each and head-of-line block
                        # whichever queue issues them). The idle "attn" PSUM
                        # slot holds the transposed blocks.
                        tps = psum.tile([128, 4, 128], FP16, tag="attn")
                        for j in range(SC // 128):
                            nc.tensor.transpose(
                                tps[:, j, :], vt[:, j * 128 : (j + 1) * 128], ident
                            )
                        b0 = kv * NBLK + sc * 4
                        nc.scalar.copy(vN[:, b0 : b0 + 4, :], tps)
                    if sc == NSC - 1 and c >= NC_TILES - QH:
                        b_stage()  # m=0 attention stages ride along
                hTc = hTc_next

            # ---- remaining pipeline stages, drain, and o_proj tail ----
            while bst["k"] < len(seq):
                b_stage()
            ph, pm, pslab, pacc = bst["st1"]
            prcpb = b_den(ph, pm, pacc)
            if bst["st2"] is not None:
                b_epilogue(*bst["st2"])
                if bst["st2"][0] == QH - 1:
                    push_c_group(bst["st2"][1])
            aps = b_attnv(ph, pm, pslab)
            b_epilogue(ph, pm, aps, prcpb)
            push_c_group(pm)
            pop_c(10 ** 9, tail=True)

    return nc


_CACHE = {}


def build_program():
    if "nc" not in _CACHE:
        nc = bacc.Bacc()
        _emit(nc)
        nc.compile()
        _CACHE["nc"] = nc
    return _CACHE["nc"]


def host_inputs(positions, hidden_states, w_qkv, w_o):
    """Build the 8 per-core input maps (host-side shard + layout + bf16 cast)."""
    positions = np.asarray(positions)
    hidden_states = np.asarray(hidden_states, dtype=np.float32)
    w_qkv = np.asarray(w_qkv, dtype=np.float32)
    w_o = np.asarray(w_o, dtype=np.float32)

    inv_freq = 1.0 / (
        ROPE_THETA ** (np.arange(0, D, 2, dtype=np.float32) / D)
    )  # [64]
    trium = np.triu(np.ones((128, 128), dtype=np.float32)).astype(np.float16)

    # per-batch tensors
    hTs, coss, sins = [], [], []
    for b in range(B):
        hT = (
            np.ascontiguousarray(hidden_states[b].T)  # [HIDDEN, S]
            .reshape(HIDDEN // 128, 128, S)
            .transpose(1, 0, 2)  # [128, ho, S]
        )
        hTs.append(np.ascontiguousarray(hT.astype(BF16)))
        ang = positions[b].astype(np.float32)[:, None] * inv_freq[None, :]  # [S,64]
        c = np.cos(ang).T  # [64, S]
        s = np.sin(ang).T
        coss.append(np.concatenate([c, c], axis=0).astype(BF16))
        sins.append(np.concatenate([s, s], axis=0).astype(BF16))

    in_maps = []
    for core in range(8):
        b, t = divmod(core, TP)
        qcols = w_qkv[:, t * QH * D : (t + 1) * QH * D]
        kcols = w_qkv[:, NH * D + t * KH * D : NH * D + (t + 1) * KH * D]
        vcols = w_qkv[:, (NH + NKV) * D + t * KH * D : (NH + NKV) * D + (t + 1) * KH * D]
        wshard = np.concatenate([qcols, kcols, vcols], axis=1)  # [4096, 1536]
        wq_t = (
            wshard.reshape(32, 128, NC_TILES, 128)
            .transpose(2, 1, 0, 3)  # [c, p, ho, m]
            .reshape(NC_TILES, 128, 32 * 128)
            .astype(BF16)
        )
        wo_shard = w_o[t * QH * D : (t + 1) * QH * D, :]  # [1024, 4096]
        wo_t = (
            wo_shard.reshape(QH, 128, 8, 512)
            .transpose(2, 1, 0, 3)  # [hc, p, co, n]
            .reshape(8, 128, 8 * 512)
            .astype(BF16)
        )
        in_maps.append(
            {
                "hT": hTs[b],
                "wq": np.ascontiguousarray(wq_t),
                "wo": np.ascontiguousarray(wo_t),
                "cosT": coss[b],
                "sinT": sins[b],
                "triuD": trium,
                "onesD": np.ones((1, 128), dtype=BF16),
                "onesMD": np.ones((128, 128), dtype=np.float16),
                "identD": np.eye(128, dtype=np.float16),
            }
        )
    return in_maps


def gather_output(results):
    """Sum the 4 TP partials per batch -> [B, S, HIDDEN] fp32."""
    outs = []
    for b in range(B):
        acc = np.zeros((S, HIDDEN), dtype=np.float32)
        for t in range(TP):
            acc += results[b * TP + t]["out"]
        outs.append(acc)
    return np.stack(outs, axis=0)


def kernel(positions, hidden_states, w_qkv, w_o, trace=False):
    nc = build_program()
    in_maps = host_inputs(positions, hidden_states, w_qkv, w_o)
    last_err = None
    for attempt in range(3):
        try:
            res = bass_utils.run_bass_kernel_spmd(
                nc, in_maps, core_ids=list(range(8)), trace=trace
            )
            break
        except Exception as e:  # transient NRT/axon device errors
            last_err = e
            import time as _time

            _time.sleep(5 * (attempt + 1))
    else:
        raise last_err
    out = gather_output(res.results)
    if trace:
        kernel.last_exec_time_ns = res.exec_time_ns
        kernel.last_results = res
    return out


# revision 40
# speedup vs baseline: 1.0238x; 1.0102x over previous
"""Trainium2 Bass kernel for Mixtral-style attention (B=2, S=2048, 32 q / 8 kv heads, D=128).

Sharding: 2-way data parallel over batch x 4-way tensor parallel over heads
(8 cores). Each core computes QKV projection for its head shard, RoPE, causal
GQA attention, and a partial o_proj (row-sharded). Host sums the 4 partials
per batch element.

All heavy matmuls run in bf16 with fp32 PSUM accumulation. Attention scores
are computed directly transposed (kT_blk^T @ qT_chunk) so exp(PSUM)->SBUF
lands straight in the probsT layout the attnT matmul needs; the causal mask
is a transposed-tril multiply on the diagonal 128x128 block only.

Softmax denominators stay OFF the PE streaming path: the DVE keeps a running
fp16 column-accumulator of the exp'd slab blocks, and a single ones[128,128]
matmul per (head, chunk) both partition-reduces the accumulator and
broadcasts the denominator to all 128 partitions in one 512-col pass
(replacing the per-block ones-row matmuls + reciprocal broadcast of the
previous version, ~85us of PE time). Phase B is a 3-stage software pipeline
(scores(k) | den+attnV(k-1) | epilogue(k-2)).

Phase C rotates PSUM across 6 banks and issues output-store DMA triggers
from the idle GpSimd queue so the Sync engine's ~600ns-per-DMA issue cost
never backs up the PSUM drain chain.
"""

import os
import sys

import numpy as np

for _p in ("/opt/trn_rl_repo", "/root/.axon_site/_ro/trn_rl_repo"):
    if os.path.isdir(_p) and _p not in sys.path:
        sys.path.insert(0, _p)

import ml_dtypes  # noqa: E402

import concourse.bass as bass  # noqa: E402
import concourse.mybir as mybir  # noqa: E402
import concourse.tile as tile  # noqa: E402
from concourse import bacc, bass_utils  # noqa: E402

BF16 = ml_dtypes.bfloat16
F32 = mybir.dt.float32
BF = mybir.dt.bfloat16
FP16 = mybir.dt.float16

B, S, HIDDEN = 2, 2048, 4096
NH, NKV, D = 32, 8, 128
TP, DP = 4, 2  # head-parallel x batch-parallel = 8 cores
QH = NH // TP  # 8 q heads per core
KH = NKV // TP  # 2 kv heads per core
NC_TILES = QH + 2 * KH  # 12 c-tiles of 128 per core (q..., k..., v...)
SC = 512  # s-chunk for phase A / attnT free dim
NSC = S // SC  # 4
NBLK = S // 128  # 16
ROPE_THETA = 10000.0
SM_SCALE = float(D) ** -0.5


def _emit(nc: bass.Bass):
    hT = nc.dram_tensor("hT", [128, HIDDEN // 128, S], BF, kind="ExternalInput")
    wq = nc.dram_tensor("wq", [NC_TILES, 128, 32 * 128], BF, kind="ExternalInput")
    wo = nc.dram_tensor("wo", [8, 128, 8 * 512], BF, kind="ExternalInput")
    cosT = nc.dram_tensor("cosT", [128, S], BF, kind="ExternalInput")
    sinT = nc.dram_tensor("sinT", [128, S], BF, kind="ExternalInput")
    triuD = nc.dram_tensor("triuD", [128, 128], FP16, kind="ExternalInput")
    onesD = nc.dram_tensor("onesD", [1, 128], BF, kind="ExternalInput")
    onesMD = nc.dram_tensor("onesMD", [128, 128], FP16, kind="ExternalInput")
    identD = nc.dram_tensor("identD", [128, 128], FP16, kind="ExternalInput")
    out = nc.dram_tensor("out", [S, HIDDEN], F32, kind="ExternalOutput")

    with tile.TileContext(nc) as tc:
        with (
            tc.tile_pool(name="const", bufs=1) as constp,
            tc.tile_pool(name="big", bufs=2) as bigp,
            tc.tile_pool(name="wt", bufs=3) as wtp,
            tc.tile_pool(name="pers", bufs=1) as pers,
            tc.tile_pool(name="rope", bufs=1) as ropep,
            tc.tile_pool(name="small", bufs=2) as smallp,
            tc.tile_pool(name="acc", bufs=2) as accp,
            tc.tile_pool(name="outp", bufs=4) as outp,
            tc.tile_pool(name="psum", bufs=2, space="PSUM") as psum,
            tc.tile_pool(name="psum_s", bufs=2, space="PSUM") as psum_s,
        ):
            # ---- startup: critical-path DMAs first, then constants ----
            ones1 = constp.tile([1, 128], BF, tag="ones1")
            nc.sync.dma_start(ones1, onesD[:])

            def load_hTc(sc, eng=None):
                # startup load rides the ACT queue (Sync is busy with wct);
                # in-loop loads ride Sync, where their ~4us strided-descriptor
                # DIRECT2D issue cannot block the rope PSUM drains on ACT.
                eng = eng or nc.sync
                t = bigp.tile([128, 32, SC], BF, tag="bigslot")
                for q in range(4):
                    eng.dma_start(
                        t[:, q * 8 : (q + 1) * 8, :],
                        hT[:, q * 8 : (q + 1) * 8, sc * SC : (sc + 1) * SC],
                    )
                return t

            def load_wct(c):
                t = wtp.tile([128, 32 * 128], BF, tag="wt")
                for q in range(4):
                    nc.sync.dma_start(
                        t[:, q * 1024 : (q + 1) * 1024],
                        wq[c, :, q * 1024 : (q + 1) * 1024],
                    )
                return t

            hTc0 = load_hTc(0, eng=nc.scalar)
            # cos/sin ride the ACT queue: on Sync they'd sit behind the wct
            # quarters and arrive after rope(c=0) needs them.
            cos_sb = constp.tile([128, S], BF, tag="cos")
            sin_sb = constp.tile([128, S], BF, tag="sin")
            nc.scalar.dma_start(cos_sb, cosT[:])
            nc.scalar.dma_start(sin_sb, sinT[:])
            wct_pre = [load_wct(0), load_wct(1), load_wct(2)]

            triu = constp.tile([128, 128], FP16, tag="triu")
            onesM = constp.tile([128, 128], FP16, tag="onesM")
            ident = constp.tile([128, 128], FP16, tag="ident")
            nc.sync.dma_start(triu, triuD[:])
            nc.sync.dma_start(onesM, onesMD[:])
            nc.sync.dma_start(ident, identD[:])

            # HAM warm-up: dummy matmuls on the tiny ones row while the first
            # hidden/weight DMAs are in flight, so the PE is already
            # un-throttled when real data arrives.
            wps = psum_s.tile([128, 2, 512], F32, tag="scores")
            for w in range(32):
                nc.tensor.matmul(
                    wps[:, 0, :128], ones1, ones1, start=(w == 0), stop=(w == 31),
                    skip_group_check=True,
                )
            dwarm = smallp.tile([128, 128], BF, tag="dwarm")
            nc.scalar.copy(dwarm, wps[:, 0, :128])

            # persistent activations
            qT = pers.tile([128, QH, S], BF, tag="qT")  # [d, head, s]
            kT = pers.tile([128, KH, S], BF, tag="kT")
            vN = pers.tile([128, KH * NBLK, 128], FP16, tag="vN")  # [sk, kv*blk, d]
            aT = pers.tile([128, QH, S], BF, tag="aT")  # [d, head, s]

            def rope_into(dst, ps, sc):
                # dst = ps * cos + rot(ps) * sin ; rot = [-x2, x1]
                rot = ropep.tile([128, SC], F32, tag="rot")
                nc.scalar.mul(rot[0:64, :], ps[64:128, :], -1.0)
                nc.scalar.copy(rot[64:128, :], ps[0:64, :])
                t2 = ropep.tile([128, SC], F32, tag="t2")
                cs = cos_sb[:, sc * SC : (sc + 1) * SC]
                sn = sin_sb[:, sc * SC : (sc + 1) * SC]
                nc.vector.tensor_mul(t2, ps, cs)
                nc.vector.tensor_mul(rot, rot, sn)
                nc.vector.tensor_add(dst, t2, rot)

            # ---- Phase B: causal GQA attention per head ----
            # slab[:, j, :] holds (unnormalized) probsT for sk-block j of the
            # current sq-chunk, in fp16. As each block is exp'd the DVE folds
            # it into a running fp16 accumulator `acc`; one ones[128,128]
            # matmul per (h, m) then partition-reduces acc AND broadcasts the
            # denominator to all 128 partitions; reciprocal + normalize are
            # DVE-only.
            def b_scores(h, m):
                kv = h // (QH // KH)
                if m == 0:
                    # m=0 slabs are tiny and come from their own pool so these
                    # stages can interleave into phase A's last chunk while
                    # bigp's two buffers still hold hTc tiles.
                    slab = accp.tile([128, 4, SC], FP16, tag="slab0")
                else:
                    slab = bigp.tile([128, NBLK, SC], FP16, tag="bigslot")
                acc = accp.tile([128, SC], FP16, tag="acc")
                qm = qT[:, h, m * 512 : (m + 1) * 512]
                for p in range(2 * m + 2):  # block pairs (2p, 2p+1)
                    j0 = 2 * p
                    diag = j0 >= 4 * m
                    pps = psum_s.tile([128, 2, 512], F32, tag="scores")
                    for u in range(2):
                        j = j0 + u
                        c0 = max(0, j - 4 * m) * 128
                        # diagonal blocks write at their ALIGNED offset so a
                        # single fused exp covers the pair; the dead columns
                        # [0, c0) hold garbage that no consumer ever reads.
                        nc.tensor.matmul(
                            pps[:, u, c0:],
                            kT[:, kv, j * 128 : (j + 1) * 128],
                            qm[:, c0:],
                            start=True,
                            stop=True,
                            skip_group_check=True,
                        )
                    nc.scalar.activation(
                        slab[:, j0 : j0 + 2, :],
                        pps,
                        mybir.ActivationFunctionType.Exp,
                        scale=SM_SCALE,
                    )
                    for u in range(2):
                        j = j0 + u
                        c0 = max(0, j - 4 * m) * 128
                        if diag:
                            blk = slab[:, j, c0 : c0 + 128]
                            nc.vector.tensor_mul(blk, blk, triu)
                        if j == 0:
                            nc.vector.tensor_copy(acc, slab[:, 0, :])
                        else:
                            nc.vector.tensor_add(
                                acc[:, c0:], acc[:, c0:], slab[:, j, c0:]
                            )
                return slab, acc

            def b_den(h, m, acc):
                # den matmul + reciprocal, emitted BEFORE the next stage's
                # DVE add-chain so the reciprocal (and the trailing epilogue
                # mul) are not queued behind ~5us of adds on the in-order DVE.
                dps = psum.tile([128, 512], F32, tag="mm512")
                nc.tensor.matmul(dps, onesM, acc, start=True, stop=True)
                rcpb = smallp.tile([128, 512], F32, tag="rcpb")
                nc.vector.reciprocal_approx_fast(rcpb, dps)
                return rcpb

            def b_attnv(h, m, slab):
                kv = h // (QH // KH)
                aps = psum.tile([128, 512], F32, tag="attn")
                for j in range(4 * m):
                    nc.tensor.matmul(
                        aps, vN[:, kv * NBLK + j, :], slab[:, j, :],
                        start=(j == 0), stop=False, skip_group_check=True,
                    )
                for jj in range(4):
                    j = 4 * m + jj
                    cs = slice(jj * 128, 512)
                    first = m == 0 and jj == 0
                    nc.tensor.matmul(
                        aps[:, cs], vN[:, kv * NBLK + j, :], slab[:, j, cs],
                        start=first, stop=(jj == 3), skip_group_check=True,
                    )
                return aps

            def b_epilogue(h, m, aps, rcpb):
                nc.vector.tensor_mul(aT[:, h, m * 512 : (m + 1) * 512], aps, rcpb)

            # ---- Phase C tiles (partial o_proj = attnT^T @ w_o_shard), ----
            # interleaved into phase B as PE filler. Once all 8 heads of
            # sequence chunk m have been normalized into aT, the 32 o_proj
            # tiles for st in [4m, 4m+4) are pushed onto a queue and drained
            # a few per pipeline stage while the next chunk's attention is
            # ACT/DVE-bound.
            from collections import deque

            c_pending = deque()
            wot_cur = {}

            def load_wot(hc):
                t = wtp.tile([128, 8 * 512], BF, tag="wt")
                for q in range(4):
                    nc.sync.dma_start(
                        t[:, q * 1024 : (q + 1) * 1024],
                        wo[hc, :, q * 1024 : (q + 1) * 1024],
                    )
                return t

            def push_c_group(m):
                # wot markers one quad early so weights prefetch ~7us ahead
                c_pending.append(("wot", 0))
                for hc in range(8):
                    if hc + 1 < 8:
                        c_pending.append(("wot", hc + 1))
                    for st in range(4 * m, 4 * m + 4):
                        c_pending.append(("tile", hc, st))

            c_rot = [0, None]  # rotation counter / current scores pair tile

            def emit_c_tile(hc, st, tail=False):
                wot = wot_cur[hc]
                if not tail:
                    ops = psum.tile([128, 512], F32, tag="mm512")
                else:
                    # after phase B drains, rotate over all free PSUM banks
                    r = c_rot[0] % 4
                    c_rot[0] += 1
                    if r == 0:
                        ops = psum.tile([128, 512], F32, tag="mm512")
                    elif r == 1:
                        ops = psum.tile([128, 512], F32, tag="attn")
                    elif r == 2:
                        cpair = psum_s.tile([128, 2, 512], F32, tag="scores")
                        c_rot[1] = cpair
                        ops = cpair[:, 0, :]
                    else:
                        ops = c_rot[1][:, 1, :]
                for cb in range(QH):
                    nc.tensor.matmul(
                        ops,
                        aT[:, cb, st * 128 : (st + 1) * 128],
                        wot[:, cb * 512 : (cb + 1) * 512],
                        start=(cb == 0),
                        stop=(cb == QH - 1),
                    )
                ot = outp.tile([128, 512], F32, tag="ot")
                nc.scalar.copy(ot, ops)
                # tail stores alternate queues so neither trigger queue's
                # issue latency paces the drain chain
                seng = nc.sync if (tail and st % 2) else nc.gpsimd
                seng.dma_start(
                    out[st * 128 : (st + 1) * 128, hc * 512 : (hc + 1) * 512], ot
                )

            def pop_c(n, tail=False):
                emitted = 0
                while c_pending:
                    item = c_pending[0]
                    if item[0] == "wot":
                        # process weight-load markers eagerly (prefetch)
                        c_pending.popleft()
                        wot_cur[item[1]] = load_wot(item[1])
                        continue
                    if emitted >= n:
                        break
                    _, hc, st = c_pending.popleft()
                    emit_c_tile(hc, st, tail=tail)
                    emitted += 1

            # 3-stage software pipeline over (chunk, head). Per stage k the
            # emission order is: den(k-1)+recip(k-1), epilogue-mul(k-2) (both
            # ahead of the new DVE add-chain), scores(k), filler, attnV(k-1),
            # filler. o_proj filler tiles are popped between stages.
            # The m=0 stages are emitted by the phase A loop (interleaved into
            # sc=3, where the PE stream hides their ACT-bound exp latency).
            seq = [(h, m) for m in range(NSC) for h in range(QH)]
            bst = {"k": 0, "st1": None, "st2": None}

            def b_stage():
                h, m = seq[bst["k"]]
                bst["k"] += 1
                st1, st2 = bst["st1"], bst["st2"]
                prcpb = None
                if st1 is not None:
                    ph, pm, pslab, pacc = st1
                    prcpb = b_den(ph, pm, pacc)
                    if st2 is not None:
                        b_epilogue(*st2)
                        if st2[0] == QH - 1:  # chunk st2[1] fully in aT
                            push_c_group(st2[1])
                            pop_c(0)  # eager wot prefetch
                slab, acc = b_scores(h, m)
                pop_c(2)
                if st1 is not None:
                    ph, pm, pslab, pacc = st1
                    aps = b_attnv(ph, pm, pslab)
                    bst["st2"] = (ph, pm, aps, prcpb)
                bst["st1"] = (h, m, slab, acc)
                pop_c(2)

            # ---- Phase A: QKV^T = w_shard^T @ hidden^T, RoPE, V transpose.
            # The 8 m=0 attention stages (which only need sc=0 outputs) are
            # interleaved into sc=3 as extra PE work to hide their ACT-bound
            # exp latency.
            hTc = hTc0
            hTc_next = None
            for sc in range(NSC):
                for c in range(NC_TILES):
                    if sc == 0 and c < 3:
                        wct = wct_pre[c]
                    else:
                        wct = load_wct(c)
                    if c == 6 and sc + 1 < NSC:
                        hTc_next = load_hTc(sc + 1)
                    ps = psum.tile([128, SC], F32, tag="mm512")
                    for ho in range(32):
                        nc.tensor.matmul(
                            ps,
                            wct[:, ho * 128 : (ho + 1) * 128],
                            hTc[:, ho, :],
                            start=(ho == 0),
                            stop=(ho == 31),
                        )
                    if c < QH:
                        rope_into(qT[:, c, sc * SC : (sc + 1) * SC], ps, sc)
                    elif c < QH + KH:
                        rope_into(kT[:, c - QH, sc * SC : (sc + 1) * SC], ps, sc)
                    else:
                        kv = c - QH - KH
                        vt = ropep.tile([128, SC], FP16, tag="vt")
                        nc.scalar.copy(vt, ps)
                        # V transpose on the PE (DMA-transpose triggers cost
                        # ~1.2us of issue time each and head-of-line block
                        # whichever queue issues them). The idle "attn" PSUM
                        # slot holds the transposed blocks.
                        tps = psum.tile([128, 4, 128], FP16, tag="attn")
                        for j in range(SC // 128):
                            nc.tensor.transpose(
                                tps[:, j, :], vt[:, j * 128 : (j + 1) * 128], ident
                            )
                        b0 = kv * NBLK + sc * 4
                        nc.scalar.copy(vN[:, b0 : b0 + 4, :], tps)
                    if sc == NSC - 1 and c >= NC_TILES - QH:
                        b_stage()  # m=0 attention stages ride along
                hTc = hTc_next

            # ---- remaining pipeline stages, drain, and o_proj tail ----
            while bst["k"] < len(seq):
                b_stage()
            ph, pm, pslab, pacc = bst["st1"]
            prcpb = b_den(ph, pm, pacc)
            if bst["st2"] is not None:
                b_epilogue(*bst["st2"])
                if bst["st2"][0] == QH - 1:
                    push_c_group(bst["st2"][1])
            aps = b_attnv(ph, pm, pslab)
            b_epilogue(ph, pm, aps, prcpb)
            push_c_group(pm)
            pop_c(10 ** 9, tail=True)

    return nc


_CACHE = {}


def build_program():
    if "nc" not in _CACHE:
        nc = bacc.Bacc()
        _emit(nc)
        nc.compile()
        _CACHE["nc"] = nc
    return _CACHE["nc"]


def host_inputs(positions, hidden_states, w_qkv, w_o):
    """Build the 8 per-core input maps (host-side shard + layout + bf16 cast)."""
    positions = np.asarray(positions)
    hidden_states = np.asarray(hidden_states, dtype=np.float32)
    w_qkv = np.asarray(w_qkv, dtype=np.float32)
    w_o = np.asarray(w_o, dtype=np.float32)

    inv_freq = 1.0 / (
        ROPE_THETA ** (np.arange(0, D, 2, dtype=np.float32) / D)
    )  # [64]
    trium = np.triu(np.ones((128, 128), dtype=np.float32)).astype(np.float16)

    # per-batch tensors
    hTs, coss, sins = [], [], []
    for b in range(B):
        hT = (
            np.ascontiguousarray(hidden_states[b].T)  # [HIDDEN, S]
            .reshape(HIDDEN // 128, 128, S)
            .transpose(1, 0, 2)  # [128, ho, S]
        )
        hTs.append(np.ascontiguousarray(hT.astype(BF16)))
        ang = positions[b].astype(np.float32)[:, None] * inv_freq[None, :]  # [S,64]
        c = np.cos(ang).T  # [64, S]
        s = np.sin(ang).T
        coss.append(np.concatenate([c, c], axis=0).astype(BF16))
        sins.append(np.concatenate([s, s], axis=0).astype(BF16))

    in_maps = []
    for core in range(8):
        b, t = divmod(core, TP)
        qcols = w_qkv[:, t * QH * D : (t + 1) * QH * D]
        kcols = w_qkv[:, NH * D + t * KH * D : NH * D + (t + 1) * KH * D]
        vcols = w_qkv[:, (NH + NKV) * D + t * KH * D : (NH + NKV) * D + (t + 1) * KH * D]
        wshard = np.concatenate([qcols, kcols, vcols], axis=1)  # [4096, 1536]
        wq_t = (
            wshard.reshape(32, 128, NC_TILES, 128)
            .transpose(2, 1, 0, 3)  # [c, p, ho, m]
            .reshape(NC_TILES, 128, 32 * 128)
            .astype(BF16)
        )
        wo_shard = w_o[t * QH * D : (t + 1) * QH * D, :]  # [1024, 4096]
        wo_t = (
            wo_shard.reshape(QH, 128, 8, 512)
            .transpose(2, 1, 0, 3)  # [hc, p, co, n]
            .reshape(8, 128, 8 * 512)
            .astype(BF16)
        )
        in_maps.append(
            {
                "hT": hTs[b],
                "wq": np.ascontiguousarray(wq_t),
                "wo": np.ascontiguousarray(wo_t),
                "cosT": coss[b],
                "sinT": sins[b],
                "triuD": trium,
                "onesD": np.ones((1, 128), dtype=BF16),
                "onesMD": np.ones((128, 128), dtype=np.float16),
                "identD": np.eye(128, dtype=np.float16),
            }
        )
    return in_maps


def gather_output(results):
    """Sum the 4 TP partials per batch -> [B, S, HIDDEN] fp32."""
    outs = []
    for b in range(B):
        acc = np.zeros((S, HIDDEN), dtype=np.float32)
        for t in range(TP):
            acc += results[b * TP + t]["out"]
        outs.append(acc)
    return np.stack(outs, axis=0)


def kernel(positions, hidden_states, w_qkv, w_o, trace=False):
    nc = build_program()
    in_maps = host_inputs(positions, hidden_states, w_qkv, w_o)
    last_err = None
    for attempt in range(3):
        try:
            res = bass_utils.run_bass_kernel_spmd(
                nc, in_maps, core_ids=list(range(8)), trace=trace
            )
            break
        except Exception as e:  # transient NRT/axon device errors
            last_err = e
            import time as _time

            _time.sleep(5 * (attempt + 1))
    else:
        raise last_err
    out = gather_output(res.results)
    if trace:
        kernel.last_exec_time_ns = res.exec_time_ns
        kernel.last_results = res
    return out
